# revision 35
# baseline (speedup 1.0000x reference)
"""Trainium2 Bass kernel for nn_CauseEffectRepertoire.

Computes, for each of 2 directions (cause/effect) and batch b:
    min over masks m of KL(full_b || 0.5*(softmax(MLP(state_b*bits_m)) +
                                          softmax(MLP(state_b*(1-bits_m)))))
with D=16, H=64, B=8, M=2^15-1=32767 masks, via an 8-core SPMD kernel that
shards the mask axis (4096 masks per core, padded with one duplicate mask).

Device math (per mask m, batch b, direction), with (b,d) = 8x16 = 128 on the
partition axis for the softmax/KL stage:
    A     = bits_m @ (state_b[:,None]*w1.T)              (mm1, row-banded K=32)
    relu_a = relu(A + b1)        relu_b = relu(C - A - b1),  C = colsum(Wb)+2*b1
    la/lb = relu @ w2.T  (+ b2 folded into Exp bias)     [fp16 stage: errors
        are 2nd-order near the KL minimum -> safe]
    Ea = exp(la + b2) (fp16: KL expr is invariant to per-mask rescale of Ea)
    Za = sum_d Ea            (block-ones matmul)
    u'_d = k_d*(Ea_d*Zb + Eb_d*Za),  k_d = 1/(2*full_d) folded into the
        Z-broadcast matmul G' so ln(u') ~ 0 near the minimum -> fp16-safe
    s = sum_d full_d*ln(u'_d) - lnZa - lnZb             (block-diag matmuls)
    device returns max over masks of s  (per b, dir)
Host: KL = (H - s + shift)/ln2 + 1; min over cores == max over s.
"""

import os
import sys
from contextlib import ExitStack

import numpy as np

sys.path.insert(0, "/opt/trn_rl_repo")

D, H, B = 16, 64, 8
M = 2 ** (D - 1) - 1  # 32767
NCORES = 8
MPAD = 32768
MC = MPAD // NCORES  # 4096 masks per core
CHUNK = 512
NCHUNK = MC // CHUNK  # 8
LN2 = float(np.log(2.0))
N_ACT_PAIRS = 2  # pairs whose relu fork runs on ScalarE instead of VectorE
USE_GPS_ADD = True  # route the u-add to GpSimd instead of VectorE
MM1_ROWBAND = True  # mm1 as 4 concurrent K=32 row-band matmuls

_f32 = np.float32
_f16 = np.float16


def _mlp_softmax_np(x, w1, b1, w2, b2):
    h = np.maximum(x @ w1.T + b1, 0.0)
    lg = h @ w2.T + b2
    lg = lg - lg.max(axis=-1, keepdims=True)
    e = np.exp(lg)
    return e / e.sum(axis=-1, keepdims=True)


def _host_prep(inputs):
    """Build all device input arrays (float64 math, float32/16 outputs)."""
    state = np.asarray(inputs["state"], dtype=np.float64)  # (B, D)
    dirs = []
    for pre in ("cause", "effect"):
        dirs.append(
            tuple(
                np.asarray(inputs[f"{pre}_{k}"], dtype=np.float64)
                for k in ("w1", "b1", "w2", "b2")
            )
        )

    # mask bits, padded to MPAD with a duplicate of mask value 1
    mv = np.concatenate([np.arange(1, M + 1, dtype=np.int64), [1]])
    bits = ((mv[:, None] >> np.arange(D)[None, :]) & 1).astype(np.float64)  # (MPAD, D)

    # per-core bitsQ (128, MC): bits.T duplicated twice per 32-row band
    bitsQ_cores = []
    for c in range(NCORES):
        bc = bits[c * MC : (c + 1) * MC].T  # (D, MC)
        band = np.concatenate([bc, bc], axis=0)  # (32, MC)
        bitsQ_cores.append(np.tile(band, (4, 1)).astype(_f16))  # (128, MC)

    # mm1 stationaries (fp16, no bias row): (128, 8*128)
    # pair p lives in row band 32p (so 4 mm1 matmuls run in parallel row bands)
    mm1w = np.zeros((128, 8 * 128))
    cvec = np.zeros((128, 8))  # ts operand: colsum(Wb) + b1
    b1t = np.zeros((128, 2))
    for d_ in range(2):
        w1, b1, w2, b2 = dirs[d_]
        b1t[0:64, d_] = b1
        b1t[64:128, d_] = b1
        for p in range(4):
            idx = d_ * 4 + p
            for half, b_ in enumerate((2 * p, 2 * p + 1)):
                Wb = state[b_][:, None] * w1.T  # (D, H)
                r0 = 32 * p + half * 16
                c0 = idx * 128 + half * 64
                mm1w[r0 : r0 + 16, c0 : c0 + 64] = Wb
                cvec[half * 64 : half * 64 + 64, idx] = Wb.sum(axis=0) + b1

    # mm2 stationaries (fp16): (128, 192): per dir 3 blocks of (128, 32):
    #   a (+w2T), bN (-w2T: DVE pairs emit -relu_b), bP (+w2T: ACT pairs)
    mm2w = np.zeros((128, 192))
    for d_ in range(2):
        w2T = dirs[d_][2].T  # (H, D)
        for blk, s in ((0, 1.0), (1, -1.0), (2, 1.0)):
            c0 = (d_ * 3 + blk) * 32
            mm2w[0:64, c0 : c0 + 16] = s * w2T
            mm2w[64:128, c0 + 16 : c0 + 32] = s * w2T

    # Z-sum lhsT (128, 32): col i sums partition block of b = i%8 (4x replicas)
    zones4 = np.zeros((128, 32))
    for k in range(128):
        for r in range(4):
            zones4[k, 8 * r + k // 16] = 1.0

    # full softmax per dir, k compression consts, G' broadcast, fmat, shifts
    fmat = np.zeros((128, 16))
    G2 = np.zeros((128, 256))
    Hc = np.zeros((2, B))
    shift = np.zeros((2, B))
    for d_ in range(2):
        w1, b1, w2, b2 = dirs[d_]
        full = _mlp_softmax_np(state, w1, b1, w2, b2)  # (B, D)
        Hc[d_] = (full * np.log(full)).sum(axis=1)
        kvec = 1.0 / (2.0 * full.reshape(-1))  # (128,) per (b,d) row
        k16 = kvec.astype(_f16).astype(np.float64)
        shift[d_] = (full * np.log(k16.reshape(B, D))).sum(axis=1)
        for b_ in range(B):
            fmat[16 * b_ : 16 * b_ + 16, d_ * 8 + b_] = full[b_]
        for j in range(128):
            for i in range(128):
                if j // 16 == i // 16:
                    G2[j, d_ * 128 + i] = k16[i]

    # lnZ correction: -lnZa[b] - lnZb[b], picking one replica row each
    fselN = np.zeros((64, 8))
    for b_ in range(B):
        fselN[b_, b_] = -1.0
        fselN[32 + b_, b_] = -1.0

    b2t = np.zeros((128, 2))
    for d_ in range(2):
        b2t[:, d_] = np.tile(dirs[d_][3], B)

    shared = {
        "mm1w": mm1w.astype(_f16),
        "cvec": cvec.astype(_f32),
        "b1t": b1t.astype(_f32),
        "mm2w": mm2w.astype(_f16),
        "zones4": zones4.astype(_f16),
        "G2": G2.astype(_f16),
        "fmat": fmat.astype(_f16),
        "fselN": fselN.astype(_f16),
        "b2t": b2t.astype(_f32),
    }
    in_maps = []
    for c in range(NCORES):
        m = dict(shared)
        m["bitsQ"] = bitsQ_cores[c]
        in_maps.append(m)
    return in_maps, Hc, shift, bits, dirs, state


def _patch_act_tables():
    """Force every activation to resolve to natural_log_exp_and_others
    (contains Ln, Exp, Relu, Copy, Identity) so the kernel pays exactly one
    ACT table load instead of one per Exp<->Ln<->Relu transition (~2.7us
    each). Set indices are preserved; other sets are just made unmatchable.
    """
    import concourse.bacc as bacc_mod
    from concourse import hw_specs

    if getattr(bacc_mod, "_act_tables_patched", False):
        return
    orig = hw_specs.get_activation_tables

    def only_nle(arch):
        t = dict(orig(arch))
        if "natural_log_exp_and_others" in t:
            t = {
                k: (v if k == "natural_log_exp_and_others" else set())
                for k, v in t.items()
            }
        return t

    bacc_mod.get_activation_tables = only_nle
    bacc_mod._act_tables_patched = True


_NC_CACHE = {}


def build_nc(repeats=1):
    """Build and compile the 8-core SPMD Bass program (cached).

    repeats>1 wraps the whole computation in a device-side loop — used only
    for benchmarking (amortizes host/tunnel dispatch overhead).
    """
    if repeats in _NC_CACHE:
        return _NC_CACHE[repeats]

    import concourse.bacc as bacc
    import concourse.bass as bass
    import concourse.tile as tile
    from concourse import mybir

    _patch_act_tables()

    AF = mybir.ActivationFunctionType
    OP = mybir.AluOpType
    f32 = mybir.dt.float32
    f16 = mybir.dt.float16

    nc = bacc.Bacc(
        "TRN2", target_bir_lowering=False, debug=False, num_devices=NCORES
    )

    ins = {}
    dts = {}
    for name, shape, dt in (
        ("bitsQ", (128, MC), f16),
        ("mm1w", (128, 8 * 128), f16),
        ("cvec", (128, 8), f32),
        ("b1t", (128, 2), f32),
        ("mm2w", (128, 192), f16),
        ("zones4", (128, 32), f16),
        ("G2", (128, 256), f16),
        ("fmat", (128, 16), f16),
        ("fselN", (64, 8), f16),
        ("b2t", (128, 2), f32),
    ):
        ins[name] = nc.dram_tensor(name, shape, dt, kind="ExternalInput").ap()
        dts[name] = dt
    # full per-mask score dump: (104, 4*CHUNK): [dir0 sg0 | dir0 sg1 | dir1 ...]
    out_d = nc.dram_tensor("sdump", (104, 4 * CHUNK), f16,
                           kind="ExternalOutput").ap()

    with tile.TileContext(nc) as tc, ExitStack() as ctx:
        cpool = ctx.enter_context(tc.tile_pool(name="consts", bufs=1))
        spool = ctx.enter_context(tc.tile_pool(name="work", bufs=2))
        rpool = ctx.enter_context(tc.tile_pool(name="relu", bufs=4))
        # PSUM budget (8 banks): l1 2 + L2 2 + z2e 2 + zc 1 + dstack 1
        pp_l1 = ctx.enter_context(tc.tile_pool(name="pl1", bufs=2, space="PSUM"))
        pp_L2 = ctx.enter_context(tc.tile_pool(name="pL2", bufs=1, space="PSUM"))
        pp_z = ctx.enter_context(tc.tile_pool(name="pz", bufs=1, space="PSUM"))
        pp_zc = ctx.enter_context(tc.tile_pool(name="pzc", bufs=1, space="PSUM"))
        pp_ds = ctx.enter_context(tc.tile_pool(name="pds", bufs=1, space="PSUM"))

        ct = {}
        for name in ins:
            shp = list(ins[name].shape)
            t = cpool.tile(shp, dts[name], tag=name, name=f"c_{name}")
            nc.sync.dma_start(t[:], ins[name][:])
            ct[name] = t

        rep_ctx = tc.For_i(0, repeats, 1) if repeats > 1 else None
        if rep_ctx is not None:
            rep_ctx.__enter__()

        sout = cpool.tile([104, 4 * CHUNK], f16, tag="sout")

        for d_ in range(2):
            dstack = None
            for n in range(NCHUNK):
                # ---- mm1 (4 parallel row bands) -> relu fork -> mm2 ----
                L2 = pp_L2.tile([128, 2 * CHUNK], f32, tag="L2")
                for p in range(4):
                    idx = d_ * 4 + p
                    lt = pp_l1.tile([128, CHUNK], f32, tag="l1")
                    if MM1_ROWBAND:
                        nc.tensor.matmul(
                            lt[:],
                            ct["mm1w"][32 * p : 32 * p + 32,
                                       idx * 128 : (idx + 1) * 128],
                            ct["bitsQ"][32 * p : 32 * p + 32,
                                        n * CHUNK : (n + 1) * CHUNK],
                            start=True, stop=True, tile_position=(32 * p, 0),
                        )
                    else:
                        nc.tensor.matmul(
                            lt[:],
                            ct["mm1w"][:, idx * 128 : (idx + 1) * 128],
                            ct["bitsQ"][:, n * CHUNK : (n + 1) * CHUNK],
                            start=True, stop=True,
                        )
                    if p < N_ACT_PAIRS:
                        # ScalarE fork: ra = relu(A+b1), rb = +relu(C-A-b1)
                        ra = rpool.tile([128, CHUNK], f16, tag="ra")
                        nc.scalar.activation(
                            ra[:], lt[:], AF.Relu,
                            bias=ct["b1t"][:, d_ : d_ + 1],
                        )
                        rb = rpool.tile([128, CHUNK], f16, tag="rb")
                        nc.scalar.activation(
                            rb[:], lt[:], AF.Relu,
                            bias=ct["cvec"][:, idx : idx + 1], scale=-1.0,
                        )
                        bblk = 2  # +w2T
                    else:
                        # VectorE fork via one fp16 copy + two 4x ts ops
                        linS = rpool.tile([128, CHUNK], f16, tag="linS")
                        nc.vector.tensor_copy(linS[:], lt[:])
                        ra = rpool.tile([128, CHUNK], f16, tag="ra")
                        nc.vector.tensor_scalar(
                            ra[:], linS[:], ct["b1t"][:, d_ : d_ + 1], 0.0,
                            OP.add, OP.max,
                        )
                        rb = rpool.tile([128, CHUNK], f16, tag="rb")
                        nc.vector.tensor_scalar(
                            rb[:], linS[:], ct["cvec"][:, idx : idx + 1], 0.0,
                            OP.subtract, OP.min,
                        )
                        bblk = 1  # -w2T (rb = -relu_b)
                    nc.tensor.matmul(
                        L2[32 * p : 32 * p + 32, 0:CHUNK],
                        ct["mm2w"][:, (d_ * 3) * 32 : (d_ * 3) * 32 + 32],
                        ra[:],
                        start=True, stop=True, tile_position=(0, 32 * p),
                    )
                    c0 = (d_ * 3 + bblk) * 32
                    nc.tensor.matmul(
                        L2[32 * p : 32 * p + 32, CHUNK : 2 * CHUNK],
                        ct["mm2w"][:, c0 : c0 + 32],
                        rb[:],
                        start=True, stop=True, tile_position=(0, 32 * p),
                    )
                # ---- exp over both sides at once (fp16 out, bias=b2) ----
                E2 = spool.tile([128, 2 * CHUNK], f16, tag="E2")
                nc.scalar.activation(
                    E2[:], L2[:], AF.Exp, bias=ct["b2t"][:, d_ : d_ + 1]
                )
                E2c = spool.tile([128, 2 * CHUNK], f32, tag="E2c")
                nc.vector.tensor_copy(E2c[:], E2[:])
                # ---- Z sums (4x replicated -> garbage-free) + lnZ ----
                ZC = pp_zc.tile([64, CHUNK], f32, tag="zc")
                nc.tensor.matmul(ZC[0:32, :], ct["zones4"][:], E2[:, 0:CHUNK],
                                 start=True, stop=True)
                nc.tensor.matmul(ZC[32:64, :], ct["zones4"][:],
                                 E2[:, CHUNK : 2 * CHUNK],
                                 start=True, stop=True, tile_position=(0, 32))
                lnZ = spool.tile([64, CHUNK], f16, tag="lnZ")
                nc.scalar.activation(lnZ[:], ZC[:], AF.Ln)
                # ---- broadcast k*Z to (b,d) rows: k*Zb | k*Za ----
                Z2e = pp_z.tile([128, 2 * CHUNK], f32, tag="z2e")
                g = ct["G2"][:, d_ * 128 : (d_ + 1) * 128]
                nc.tensor.matmul(Z2e[:, 0:CHUNK], g,
                                 E2[:, CHUNK : 2 * CHUNK], start=True, stop=True)
                nc.tensor.matmul(Z2e[:, CHUNK : 2 * CHUNK], g,
                                 E2[:, 0:CHUNK], start=True, stop=True)
                # ---- u' = k*(Ea*Zb + Eb*Za) ----
                uu = spool.tile([128, 2 * CHUNK], f32, tag="uu")
                nc.vector.tensor_mul(uu[:], E2c[:], Z2e[:])
                u = spool.tile([128, CHUNK], f32, tag="u")
                if USE_GPS_ADD:
                    nc.gpsimd.tensor_tensor(
                        u[:], uu[:, 0:CHUNK], uu[:, CHUNK : 2 * CHUNK], OP.add
                    )
                else:
                    nc.vector.tensor_add(
                        u[:], uu[:, 0:CHUNK], uu[:, CHUNK : 2 * CHUNK]
                    )
                lnU = spool.tile([128, CHUNK], f16, tag="lnU")
                nc.scalar.activation(lnU[:], u[:], AF.Ln)
                # ---- dot with full + lnZ correction into dstack slot ----
                s_ = n % 4
                if s_ == 0:
                    dstack = pp_ds.tile([104, CHUNK], f32, tag="ds")
                nc.tensor.matmul(dstack[32 * s_ : 32 * s_ + 8, :],
                                 ct["fmat"][:, d_ * 8 : d_ * 8 + 8],
                                 lnU[:], start=True, stop=False,
                                 tile_position=(0, 32 * s_))
                nc.tensor.matmul(dstack[32 * s_ : 32 * s_ + 8, :],
                                 ct["fselN"][:], lnZ[:],
                                 start=False, stop=True,
                                 tile_position=(0, 32 * s_))
                if s_ == 3:
                    sg = n // 4
                    nc.vector.tensor_copy(
                        sout[:, (d_ * 2 + sg) * CHUNK : (d_ * 2 + sg + 1) * CHUNK],
                        dstack[:],
                    )

        nc.sync.dma_start(out_d[:], sout[:])

        if rep_ctx is not None:
            rep_ctx.__exit__(None, None, None)

    nc.compile()
    _NC_CACHE[repeats] = nc
    return nc


DELTA = 0.02  # nats: candidate margin below the device max (>> fp16 noise)


def kernel(**inputs):
    from concourse.bass_utils import run_bass_kernel_spmd

    in_maps, Hc, shift, bits, dirs, state = _host_prep(inputs)
    nc = build_nc()
    res = run_bass_kernel_spmd(nc, in_maps, list(range(NCORES)))
    sd = np.stack([r["sdump"].astype(np.float32) for r in res.results])

    # reassemble s~[dir, mask, b] from (core, 104, 4*CHUNK) slot layout
    s_all = np.empty((2, MPAD, B), np.float32)
    blk = sd.reshape(NCORES, 104, 4, CHUNK)  # (core, row, d*2+sg, c)
    for d_ in range(2):
        for sg in range(2):
            for s_ in range(4):
                rows = blk[:, 32 * s_ : 32 * s_ + 8, d_ * 2 + sg, :]  # (core,8,c)
                n = sg * 4 + s_
                for c_ in range(NCORES):
                    m0 = c_ * MC + n * CHUNK
                    s_all[d_, m0 : m0 + CHUNK, :] = rows[c_].T

    # exact float64 re-evaluation of near-max candidates
    out = np.zeros((2, B))
    st = state  # (B, D) float64
    for d_ in range(2):
        sm = s_all[d_]  # (MPAD, B)
        thr = sm.max(axis=0) - DELTA
        cand = np.where((sm >= thr[None, :]).any(axis=1))[0]
        bsel = bits[cand]  # (K, D)
        w1, b1, w2, b2 = dirs[d_]

        def mlp(x):
            h = np.maximum(x @ w1.T + b1, 0.0)
            lg = h @ w2.T + b2
            lg = lg - lg.max(axis=-1, keepdims=True)
            e = np.exp(lg)
            return e / e.sum(axis=-1, keepdims=True)

        full = mlp(st)  # (B, D)
        sa = mlp(st[None, :, :] * bsel[:, None, :])  # (K, B, D)
        sb = mlp(st[None, :, :] * (1.0 - bsel)[:, None, :])
        mix = 0.5 * (sa + sb)
        kl = (full[None] * (np.log2(full[None]) - np.log2(mix))).sum(-1)  # (K, B)
        out[d_] = kl.min(axis=0)
    return out.astype(np.float32)


if __name__ == "__main__":
    import reference

    inp = reference.setup_inputs()
    inp = {k: np.asarray(v) for k, v in inp.items()}
    out = kernel(**inp)
    print(out)


# revision 36
# speedup vs baseline: 1.8639x; 1.8639x over previous
"""Trainium2 Bass kernel for nn_CauseEffectRepertoire.

Computes, for each of 2 directions (cause/effect) and batch b:
    min over masks m of KL(full_b || 0.5*(softmax(MLP(state_b*bits_m)) +
                                          softmax(MLP(state_b*(1-bits_m)))))
with D=16, H=64, B=8, M=2^15-1=32767 masks, via an 8-core SPMD kernel that
shards the mask axis (4096 masks per core, padded with one duplicate mask).

Device math (per mask m, batch b, direction), with (b,d) = 8x16 = 128 on the
partition axis for the softmax/KL stage:
    A     = bits_m @ (state_b[:,None]*w1.T)              (mm1, row-banded K=32)
    relu_a = relu(A + b1)        relu_b = relu(C - A - b1),  C = colsum(Wb)+2*b1
    la/lb = relu @ w2.T  (+ b2 folded into Exp bias)     [fp16 stage: errors
        are 2nd-order near the KL minimum -> safe]
    Ea = exp(la + b2) (fp16: KL expr is invariant to per-mask rescale of Ea)
    Za = sum_d Ea            (block-ones matmul)
    u'_d = k_d*(Ea_d*Zb + Eb_d*Za),  k_d = 1/(2*full_d) folded into the
        Z-broadcast matmul G' so ln(u') ~ 0 near the minimum -> fp16-safe
    s = sum_d full_d*ln(u'_d) - lnZa - lnZb             (block-diag matmuls)
    device returns max over masks of s  (per b, dir)
Host: KL = (H - s + shift)/ln2 + 1; min over cores == max over s.
"""

import os
import sys
from contextlib import ExitStack

import numpy as np

sys.path.insert(0, "/opt/trn_rl_repo")

D, H, B = 16, 64, 8
M = 2 ** (D - 1) - 1  # 32767
NCORES = 8
MPAD = 32768
MC = MPAD // NCORES  # 4096 masks per core
CHUNK = 512
NCHUNK = MC // CHUNK  # 8
LN2 = float(np.log(2.0))
N_ACT_PAIRS = 2  # pairs whose relu fork runs on ScalarE instead of VectorE
USE_GPS_ADD = False  # route the u-add to GpSimd instead of VectorE
MM1_ROWBAND = False  # mm1 as 4 concurrent K=32 row-band matmuls

_f32 = np.float32
_f16 = np.float16


def _mlp_softmax_np(x, w1, b1, w2, b2):
    h = np.maximum(x @ w1.T + b1, 0.0)
    lg = h @ w2.T + b2
    lg = lg - lg.max(axis=-1, keepdims=True)
    e = np.exp(lg)
    return e / e.sum(axis=-1, keepdims=True)


def _host_prep(inputs):
    """Build all device input arrays (float64 math, float32/16 outputs)."""
    state = np.asarray(inputs["state"], dtype=np.float64)  # (B, D)
    dirs = []
    for pre in ("cause", "effect"):
        dirs.append(
            tuple(
                np.asarray(inputs[f"{pre}_{k}"], dtype=np.float64)
                for k in ("w1", "b1", "w2", "b2")
            )
        )

    # mask bits, padded to MPAD with a duplicate of mask value 1
    mv = np.concatenate([np.arange(1, M + 1, dtype=np.int64), [1]])
    bits = ((mv[:, None] >> np.arange(D)[None, :]) & 1).astype(np.float64)  # (MPAD, D)

    # per-core bitsQ (128, MC): bits.T duplicated twice per 32-row band
    bitsQ_cores = []
    for c in range(NCORES):
        bc = bits[c * MC : (c + 1) * MC].T  # (D, MC)
        band = np.concatenate([bc, bc], axis=0)  # (32, MC)
        bitsQ_cores.append(np.tile(band, (4, 1)).astype(_f16))  # (128, MC)

    # mm1 stationaries (fp16, no bias row): (128, 8*128)
    # pair p lives in row band 32p (so 4 mm1 matmuls run in parallel row bands)
    mm1w = np.zeros((128, 8 * 128))
    cvec = np.zeros((128, 8))  # ts operand: colsum(Wb) + b1
    b1t = np.zeros((128, 2))
    for d_ in range(2):
        w1, b1, w2, b2 = dirs[d_]
        b1t[0:64, d_] = b1
        b1t[64:128, d_] = b1
        for p in range(4):
            idx = d_ * 4 + p
            for half, b_ in enumerate((2 * p, 2 * p + 1)):
                Wb = state[b_][:, None] * w1.T  # (D, H)
                r0 = 32 * p + half * 16
                c0 = idx * 128 + half * 64
                mm1w[r0 : r0 + 16, c0 : c0 + 64] = Wb
                cvec[half * 64 : half * 64 + 64, idx] = Wb.sum(axis=0) + b1

    # mm2 stationaries (fp16): (128, 192): per dir 3 blocks of (128, 32):
    #   a (+w2T), bN (-w2T: DVE pairs emit -relu_b), bP (+w2T: ACT pairs)
    mm2w = np.zeros((128, 192))
    for d_ in range(2):
        w2T = dirs[d_][2].T  # (H, D)
        for blk, s in ((0, 1.0), (1, -1.0), (2, 1.0)):
            c0 = (d_ * 3 + blk) * 32
            mm2w[0:64, c0 : c0 + 16] = s * w2T
            mm2w[64:128, c0 + 16 : c0 + 32] = s * w2T

    # Z-sum lhsT (128, 32): col i sums partition block of b = i%8 (4x replicas)
    zones4 = np.zeros((128, 32))
    for k in range(128):
        for r in range(4):
            zones4[k, 8 * r + k // 16] = 1.0

    # full softmax per dir, k compression consts, G' broadcast, fmat, shifts
    fmat = np.zeros((128, 16))
    G2 = np.zeros((128, 256))
    Hc = np.zeros((2, B))
    shift = np.zeros((2, B))
    for d_ in range(2):
        w1, b1, w2, b2 = dirs[d_]
        full = _mlp_softmax_np(state, w1, b1, w2, b2)  # (B, D)
        Hc[d_] = (full * np.log(full)).sum(axis=1)
        kvec = 1.0 / (2.0 * full.reshape(-1))  # (128,) per (b,d) row
        k16 = kvec.astype(_f16).astype(np.float64)
        shift[d_] = (full * np.log(k16.reshape(B, D))).sum(axis=1)
        for b_ in range(B):
            fmat[16 * b_ : 16 * b_ + 16, d_ * 8 + b_] = full[b_]
        for j in range(128):
            for i in range(128):
                if j // 16 == i // 16:
                    G2[j, d_ * 128 + i] = k16[i]

    # lnZ correction: -lnZa[b] - lnZb[b], picking one replica row each
    fselN = np.zeros((64, 8))
    for b_ in range(B):
        fselN[b_, b_] = -1.0
        fselN[32 + b_, b_] = -1.0

    b2t = np.zeros((128, 2))
    for d_ in range(2):
        b2t[:, d_] = np.tile(dirs[d_][3], B)

    shared = {
        "mm1w": mm1w.astype(_f16),
        "cvec": cvec.astype(_f32),
        "b1t": b1t.astype(_f32),
        "mm2w": mm2w.astype(_f16),
        "zones4": zones4.astype(_f16),
        "G2": G2.astype(_f16),
        "fmat": fmat.astype(_f16),
        "fselN": fselN.astype(_f16),
        "b2t": b2t.astype(_f32),
    }
    in_maps = []
    for c in range(NCORES):
        m = dict(shared)
        m["bitsQ"] = bitsQ_cores[c]
        in_maps.append(m)
    return in_maps, Hc, shift, bits, dirs, state


def _patch_act_tables():
    """Force every activation to resolve to natural_log_exp_and_others
    (contains Ln, Exp, Relu, Copy, Identity) so the kernel pays exactly one
    ACT table load instead of one per Exp<->Ln<->Relu transition (~2.7us
    each). Set indices are preserved; other sets are just made unmatchable.
    """
    import concourse.bacc as bacc_mod
    from concourse import hw_specs

    if getattr(bacc_mod, "_act_tables_patched", False):
        return
    orig = hw_specs.get_activation_tables

    def only_nle(arch):
        t = dict(orig(arch))
        if "natural_log_exp_and_others" in t:
            t = {
                k: (v if k == "natural_log_exp_and_others" else set())
                for k, v in t.items()
            }
        return t

    bacc_mod.get_activation_tables = only_nle
    bacc_mod._act_tables_patched = True


_NC_CACHE = {}


def build_nc(repeats=1):
    """Build and compile the 8-core SPMD Bass program (cached).

    repeats>1 wraps the whole computation in a device-side loop — used only
    for benchmarking (amortizes host/tunnel dispatch overhead).
    """
    if repeats in _NC_CACHE:
        return _NC_CACHE[repeats]

    import concourse.bacc as bacc
    import concourse.bass as bass
    import concourse.tile as tile
    from concourse import mybir

    _patch_act_tables()

    AF = mybir.ActivationFunctionType
    OP = mybir.AluOpType
    f32 = mybir.dt.float32
    f16 = mybir.dt.float16

    nc = bacc.Bacc(
        "TRN2", target_bir_lowering=False, debug=False, num_devices=NCORES
    )

    ins = {}
    dts = {}
    for name, shape, dt in (
        ("bitsQ", (128, MC), f16),
        ("mm1w", (128, 8 * 128), f16),
        ("cvec", (128, 8), f32),
        ("b1t", (128, 2), f32),
        ("mm2w", (128, 192), f16),
        ("zones4", (128, 32), f16),
        ("G2", (128, 256), f16),
        ("fmat", (128, 16), f16),
        ("fselN", (64, 8), f16),
        ("b2t", (128, 2), f32),
    ):
        ins[name] = nc.dram_tensor(name, shape, dt, kind="ExternalInput").ap()
        dts[name] = dt
    # full per-mask score dump: (104, 4*CHUNK): [dir0 sg0 | dir0 sg1 | dir1 ...]
    out_d = nc.dram_tensor("sdump", (104, 4 * CHUNK), f16,
                           kind="ExternalOutput").ap()

    with tile.TileContext(nc) as tc, ExitStack() as ctx:
        cpool = ctx.enter_context(tc.tile_pool(name="consts", bufs=1))
        spool = ctx.enter_context(tc.tile_pool(name="work", bufs=2))
        rpool = ctx.enter_context(tc.tile_pool(name="relu", bufs=4))
        # PSUM budget (8 banks): l1 2 + L2 2 + z2e 2 + zc 1 + dstack 1
        pp_l1 = ctx.enter_context(tc.tile_pool(name="pl1", bufs=2, space="PSUM"))
        pp_L2 = ctx.enter_context(tc.tile_pool(name="pL2", bufs=1, space="PSUM"))
        pp_z = ctx.enter_context(tc.tile_pool(name="pz", bufs=1, space="PSUM"))
        pp_zc = ctx.enter_context(tc.tile_pool(name="pzc", bufs=1, space="PSUM"))
        pp_ds = ctx.enter_context(tc.tile_pool(name="pds", bufs=1, space="PSUM"))

        ct = {}
        for name in ins:
            shp = list(ins[name].shape)
            t = cpool.tile(shp, dts[name], tag=name, name=f"c_{name}")
            nc.sync.dma_start(t[:], ins[name][:])
            ct[name] = t

        rep_ctx = tc.For_i(0, repeats, 1) if repeats > 1 else None
        if rep_ctx is not None:
            rep_ctx.__enter__()

        sout = cpool.tile([104, 4 * CHUNK], f16, tag="sout")

        for d_ in range(2):
            dstack = None
            for n in range(NCHUNK):
                # ---- mm1 (4 parallel row bands) -> relu fork -> mm2 ----
                L2 = pp_L2.tile([128, 2 * CHUNK], f32, tag="L2")
                for p in range(4):
                    idx = d_ * 4 + p
                    lt = pp_l1.tile([128, CHUNK], f32, tag="l1")
                    if MM1_ROWBAND:
                        nc.tensor.matmul(
                            lt[:],
                            ct["mm1w"][32 * p : 32 * p + 32,
                                       idx * 128 : (idx + 1) * 128],
                            ct["bitsQ"][32 * p : 32 * p + 32,
                                        n * CHUNK : (n + 1) * CHUNK],
                            start=True, stop=True, tile_position=(32 * p, 0),
                        )
                    else:
                        nc.tensor.matmul(
                            lt[:],
                            ct["mm1w"][:, idx * 128 : (idx + 1) * 128],
                            ct["bitsQ"][:, n * CHUNK : (n + 1) * CHUNK],
                            start=True, stop=True,
                        )
                    if p < N_ACT_PAIRS:
                        # ScalarE fork: ra = relu(A+b1), rb = +relu(C-A-b1)
                        ra = rpool.tile([128, CHUNK], f16, tag="ra")
                        nc.scalar.activation(
                            ra[:], lt[:], AF.Relu,
                            bias=ct["b1t"][:, d_ : d_ + 1],
                        )
                        rb = rpool.tile([128, CHUNK], f16, tag="rb")
                        nc.scalar.activation(
                            rb[:], lt[:], AF.Relu,
                            bias=ct["cvec"][:, idx : idx + 1], scale=-1.0,
                        )
                        bblk = 2  # +w2T
                    else:
                        # VectorE fork via one fp16 copy + two 4x ts ops
                        linS = rpool.tile([128, CHUNK], f16, tag="linS")
                        nc.vector.tensor_copy(linS[:], lt[:])
                        ra = rpool.tile([128, CHUNK], f16, tag="ra")
                        nc.vector.tensor_scalar(
                            ra[:], linS[:], ct["b1t"][:, d_ : d_ + 1], 0.0,
                            OP.add, OP.max,
                        )
                        rb = rpool.tile([128, CHUNK], f16, tag="rb")
                        nc.vector.tensor_scalar(
                            rb[:], linS[:], ct["cvec"][:, idx : idx + 1], 0.0,
                            OP.subtract, OP.min,
                        )
                        bblk = 1  # -w2T (rb = -relu_b)
                    nc.tensor.matmul(
                        L2[32 * p : 32 * p + 32, 0:CHUNK],
                        ct["mm2w"][:, (d_ * 3) * 32 : (d_ * 3) * 32 + 32],
                        ra[:],
                        start=True, stop=True, tile_position=(0, 32 * p),
                    )
                    c0 = (d_ * 3 + bblk) * 32
                    nc.tensor.matmul(
                        L2[32 * p : 32 * p + 32, CHUNK : 2 * CHUNK],
                        ct["mm2w"][:, c0 : c0 + 32],
                        rb[:],
                        start=True, stop=True, tile_position=(0, 32 * p),
                    )
                # ---- exp over both sides at once (fp16 out, bias=b2) ----
                E2 = spool.tile([128, 2 * CHUNK], f16, tag="E2")
                nc.scalar.activation(
                    E2[:], L2[:], AF.Exp, bias=ct["b2t"][:, d_ : d_ + 1]
                )
                E2c = spool.tile([128, 2 * CHUNK], f32, tag="E2c")
                nc.vector.tensor_copy(E2c[:], E2[:])
                # ---- Z sums (4x replicated -> garbage-free) + lnZ ----
                ZC = pp_zc.tile([64, CHUNK], f32, tag="zc")
                nc.tensor.matmul(ZC[0:32, :], ct["zones4"][:], E2[:, 0:CHUNK],
                                 start=True, stop=True)
                nc.tensor.matmul(ZC[32:64, :], ct["zones4"][:],
                                 E2[:, CHUNK : 2 * CHUNK],
                                 start=True, stop=True, tile_position=(0, 32))
                lnZ = spool.tile([64, CHUNK], f16, tag="lnZ")
                nc.scalar.activation(lnZ[:], ZC[:], AF.Ln)
                # ---- broadcast k*Z to (b,d) rows: k*Zb | k*Za ----
                Z2e = pp_z.tile([128, 2 * CHUNK], f32, tag="z2e")
                g = ct["G2"][:, d_ * 128 : (d_ + 1) * 128]
                nc.tensor.matmul(Z2e[:, 0:CHUNK], g,
                                 E2[:, CHUNK : 2 * CHUNK], start=True, stop=True)
                nc.tensor.matmul(Z2e[:, CHUNK : 2 * CHUNK], g,
                                 E2[:, 0:CHUNK], start=True, stop=True)
                # ---- u' = k*(Ea*Zb + Eb*Za) ----
                uu = spool.tile([128, 2 * CHUNK], f32, tag="uu")
                nc.vector.tensor_mul(uu[:], E2c[:], Z2e[:])
                u = spool.tile([128, CHUNK], f32, tag="u")
                if USE_GPS_ADD:
                    nc.gpsimd.tensor_tensor(
                        u[:], uu[:, 0:CHUNK], uu[:, CHUNK : 2 * CHUNK], OP.add
                    )
                else:
                    nc.vector.tensor_add(
                        u[:], uu[:, 0:CHUNK], uu[:, CHUNK : 2 * CHUNK]
                    )
                lnU = spool.tile([128, CHUNK], f16, tag="lnU")
                nc.scalar.activation(lnU[:], u[:], AF.Ln)
                # ---- dot with full + lnZ correction into dstack slot ----
                s_ = n % 4
                if s_ == 0:
                    dstack = pp_ds.tile([104, CHUNK], f32, tag="ds")
                nc.tensor.matmul(dstack[32 * s_ : 32 * s_ + 8, :],
                                 ct["fmat"][:, d_ * 8 : d_ * 8 + 8],
                                 lnU[:], start=True, stop=False,
                                 tile_position=(0, 32 * s_))
                nc.tensor.matmul(dstack[32 * s_ : 32 * s_ + 8, :],
                                 ct["fselN"][:], lnZ[:],
                                 start=False, stop=True,
                                 tile_position=(0, 32 * s_))
                if s_ == 3:
                    sg = n // 4
                    nc.vector.tensor_copy(
                        sout[:, (d_ * 2 + sg) * CHUNK : (d_ * 2 + sg + 1) * CHUNK],
                        dstack[:],
                    )

        nc.sync.dma_start(out_d[:], sout[:])

        if rep_ctx is not None:
            rep_ctx.__exit__(None, None, None)

    nc.compile()
    _NC_CACHE[repeats] = nc
    return nc


DELTA = 0.02  # nats: candidate margin below the device max (>> fp16 noise)


def kernel(**inputs):
    from concourse.bass_utils import run_bass_kernel_spmd

    in_maps, Hc, shift, bits, dirs, state = _host_prep(inputs)
    nc = build_nc()
    res = run_bass_kernel_spmd(nc, in_maps, list(range(NCORES)))
    sd = np.stack([r["sdump"].astype(np.float32) for r in res.results])

    # reassemble s~[dir, mask, b] from (core, 104, 4*CHUNK) slot layout
    s_all = np.empty((2, MPAD, B), np.float32)
    blk = sd.reshape(NCORES, 104, 4, CHUNK)  # (core, row, d*2+sg, c)
    for d_ in range(2):
        for sg in range(2):
            for s_ in range(4):
                rows = blk[:, 32 * s_ : 32 * s_ + 8, d_ * 2 + sg, :]  # (core,8,c)
                n = sg * 4 + s_
                for c_ in range(NCORES):
                    m0 = c_ * MC + n * CHUNK
                    s_all[d_, m0 : m0 + CHUNK, :] = rows[c_].T

    # exact float64 re-evaluation of near-max candidates
    out = np.zeros((2, B))
    st = state  # (B, D) float64
    for d_ in range(2):
        sm = s_all[d_]  # (MPAD, B)
        thr = sm.max(axis=0) - DELTA
        cand = np.where((sm >= thr[None, :]).any(axis=1))[0]
        bsel = bits[cand]  # (K, D)
        w1, b1, w2, b2 = dirs[d_]

        def mlp(x):
            h = np.maximum(x @ w1.T + b1, 0.0)
            lg = h @ w2.T + b2
            lg = lg - lg.max(axis=-1, keepdims=True)
            e = np.exp(lg)
            return e / e.sum(axis=-1, keepdims=True)

        full = mlp(st)  # (B, D)
        sa = mlp(st[None, :, :] * bsel[:, None, :])  # (K, B, D)
        sb = mlp(st[None, :, :] * (1.0 - bsel)[:, None, :])
        mix = 0.5 * (sa + sb)
        kl = (full[None] * (np.log2(full[None]) - np.log2(mix))).sum(-1)  # (K, B)
        out[d_] = kl.min(axis=0)
    return out.astype(np.float32)


if __name__ == "__main__":
    import reference

    inp = reference.setup_inputs()
    inp = {k: np.asarray(v) for k, v in inp.items()}
    out = kernel(**inp)
    print(out)
